# Initial kernel scaffold
#
"""MoE layer (dense all-expert compute + top-2 routing) on 8 TRN2 NeuronCores.

Sharding: token-parallel. x has N = 4*2048 = 8192 tokens; each core gets 1024
tokens and computes all 6 experts for them (the reference computes every
expert densely for every token, so expert-parallel would save nothing and
leave cores idle with E=6 on 8 cores). No collectives needed: each core
writes its own token-slice of the output.

Per-core device program (T=1024 tokens, H=1024, F=4096, E=6):
  gate  = x @ Wg + bg              (fp32 PE matmuls -> exact top-2 selection)
  w[t,e] = top-2 softmax weights scattered to expert slots (DVE mask ops)
  out   = sum_e w[:,e] * (gelu(x @ W1[e] + b1[e]) @ W2[e]) + w @ b2
Big matmuls run in bf16 (full PE rate, fp32 PSUM accumulation); the first
matmul produces hT in [F-partition, token] layout so the second needs no
transpose.
"""

import numpy as np
import ml_dtypes

import concourse.bass as bass
import concourse.mybir as mybir
from concourse.tile import TileContext
from concourse.bass_utils import run_bass_kernel_spmd
from concourse.masks import make_identity

F32 = mybir.dt.float32
BF16 = mybir.dt.bfloat16
AF = mybir.ActivationFunctionType
ALU = mybir.AluOpType

NCORES = 8
B, S, H, F, E = 4, 2048, 1024, 4096, 6
N = B * S                    # 8192 tokens
T = N // NCORES              # 1024 tokens per core
HC = H // 128                # 8 h-chunks
FC = F // 128                # 32 f-chunks
TC = T // 128                # 8 token tiles per core


def _split_multi_waits(nc, max_waits=1):
    """The walrus build in this env accepts only one sync-wait per
    instruction; hoist extra waits onto same-engine NOPs inserted before."""
    for f in nc.m.functions:
        for bb in f.blocks:
            new = []
            dirty = False
            for inst in bb.instructions:
                si = inst.sync_info
                waits = list(si.on_wait) if si else []
                if len(waits) > max_waits:
                    dirty = True
                    for j, w in enumerate(waits[max_waits:]):
                        nop = mybir.InstNoOp(
                            name=f"{inst.name}-wsplit{j}", ins=[], outs=[]
                        )
                        nop.engine = inst.engine
                        nop.sync_info = mybir.SyncInfo(on_wait=[w], on_update=[])
                        new.append(nop)
                    inst.sync_info = mybir.SyncInfo(
                        on_wait=waits[:max_waits], on_update=list(si.on_update)
                    )
                new.append(inst)
            if dirty:
                bb.instructions = new


def _build_nc():
    nc = bass.Bass("TRN2", target_bir_lowering=False, debug=False)

    xt_bf = nc.dram_tensor("xt_bf", [HC, 128, T], BF16, kind="ExternalInput")
    xt_f32 = nc.dram_tensor("xt_f32", [HC, 128, T], F32, kind="ExternalInput")
    wg_d = nc.dram_tensor("wg", [HC, 128, E], F32, kind="ExternalInput")
    bg_d = nc.dram_tensor("bg", [E], F32, kind="ExternalInput")
    w1_d = nc.dram_tensor("w1", [E, FC, 128, HC, 128], BF16, kind="ExternalInput")
    b1_d = nc.dram_tensor("b1", [E, FC, 128], F32, kind="ExternalInput")
    w2_d = nc.dram_tensor("w2", [E, FC, 128, H], BF16, kind="ExternalInput")
    b2_d = nc.dram_tensor("b2", [E, H], F32, kind="ExternalInput")
    out_d = nc.dram_tensor("out", [T, H], F32, kind="ExternalOutput")

    with TileContext(nc) as tc:
        with (
            tc.tile_pool(name="const", bufs=1) as const,
            tc.tile_pool(name="acc", bufs=1) as accp,
        ):
            # resident tiles
            xbf = const.tile([128, HC, T], BF16)           # 16 KB/part
            nc.sync.dma_start(out=xbf, in_=xt_bf.rearrange("c p t -> p c t"))
            b1_sb = const.tile([128, E, FC], F32)          # 768 B/part
            nc.gpsimd.dma_start(out=b1_sb, in_=b1_d.rearrange("e f p -> p e f"))
            bg_sb = const.tile([128, E], F32)
            nc.gpsimd.dma_start(
                out=bg_sb,
                in_=bass.AP(tensor=bg_d.tensor, offset=0, ap=[[0, 128], [1, E]]),
            )
            b2_sb = const.tile([E, H], F32)
            nc.gpsimd.dma_start(out=b2_sb, in_=b2_d[:, :])
            ident = const.tile([128, 128], F32)
            make_identity(nc, ident)
            wsb = const.tile([128, TC, E], F32)            # routing weights
            wt_sb = const.tile([E, T], F32)                # transposed routing weights
            out_acc = accp.tile([128, TC, H], F32)         # 32 KB/part

            # ---------------- gate + routing ----------------
            with (
                tc.tile_pool(name="gx", bufs=1) as gx,
                tc.tile_pool(name="gps", bufs=2, space="PSUM") as gps,
                tc.tile_pool(name="grt", bufs=4) as grt,
                tc.tile_pool(name="gwt", bufs=2, space="PSUM") as gwt,
            ):
                xf32 = gx.tile([128, HC, T], F32)          # 32 KB/part, freed after
                nc.sync.dma_start(out=xf32, in_=xt_f32.rearrange("c p t -> p c t"))
                wg_sb = grt.tile([128, HC, E], F32)
                nc.gpsimd.dma_start(out=wg_sb, in_=wg_d.rearrange("c p e -> p c e"))

                for c in range(TC):
                    gp = gps.tile([128, E], F32)
                    for hc in range(HC):
                        nc.tensor.matmul(
                            gp,
                            lhsT=xf32[:, hc, c * 128 : (c + 1) * 128],
                            rhs=wg_sb[:, hc, :],
                            start=(hc == 0),
                            stop=(hc == HC - 1),
                        )
                    g_t = grt.tile([128, E], F32)
                    nc.vector.tensor_add(g_t, gp, bg_sb)
                    m1 = grt.tile([128, 1], F32)
                    nc.vector.tensor_reduce(
                        m1, g_t, axis=mybir.AxisListType.X, op=ALU.max
                    )
                    is1 = grt.tile([128, E], F32)
                    nc.vector.tensor_scalar(is1, g_t, m1[:, :], None, op0=ALU.is_ge)
                    g2 = grt.tile([128, E], F32)
                    nc.vector.scalar_tensor_tensor(
                        out=g2, in0=is1, scalar=-1e30, in1=g_t,
                        op0=ALU.mult, op1=ALU.add,
                    )
                    m2 = grt.tile([128, 1], F32)
                    nc.vector.tensor_reduce(
                        m2, g2, axis=mybir.AxisListType.X, op=ALU.max
                    )
                    is2 = grt.tile([128, E], F32)
                    nc.vector.tensor_scalar(is2, g2, m2[:, :], None, op0=ALU.is_ge)
                    negm1 = grt.tile([128, 1], F32)
                    nc.vector.tensor_scalar_mul(negm1, m1, -1.0)
                    e2 = grt.tile([128, 1], F32)
                    nc.scalar.activation(
                        out=e2, in_=m2, func=AF.Exp, bias=negm1[:, :], scale=1.0
                    )
                    denom = grt.tile([128, 1], F32)
                    nc.vector.tensor_scalar_add(denom, e2, 1.0)
                    winv = grt.tile([128, 1], F32)
                    nc.vector.reciprocal(winv, denom)
                    w2nd = grt.tile([128, 1], F32)
                    nc.vector.tensor_mul(w2nd, e2, winv)
                    nc.vector.tensor_scalar_mul(wsb[:, c, :], is1, winv[:, :])
                    nc.vector.scalar_tensor_tensor(
                        out=wsb[:, c, :], in0=is2, scalar=w2nd[:, :],
                        in1=wsb[:, c, :], op0=ALU.mult, op1=ALU.add,
                    )
                    # transpose w tile -> wT [E, 128]
                    wt_ps = gwt.tile([E, 128], F32)
                    nc.tensor.transpose(wt_ps, wsb[:, c, :], ident)
                    nc.vector.tensor_copy(wt_sb[:, c * 128 : (c + 1) * 128], wt_ps)

                # out_acc init = w @ b2  (handles arbitrary b2; zeros in practice)
                for c in range(TC):
                    for half in range(2):
                        po = gps.tile([128, 512], F32)
                        nc.tensor.matmul(
                            po,
                            lhsT=wt_sb[:, c * 128 : (c + 1) * 128],
                            rhs=b2_sb[:, half * 512 : (half + 1) * 512],
                            start=True,
                            stop=True,
                        )
                        nc.scalar.activation(
                            out=out_acc[:, c, half * 512 : (half + 1) * 512],
                            in_=po, func=AF.Copy,
                        )

            # ---------------- main expert loop ----------------
            with (
                tc.tile_pool(name="w1p", bufs=3) as w1p,
                tc.tile_pool(name="w2p", bufs=4) as w2p,
                tc.tile_pool(name="ht", bufs=1) as htp,
                tc.tile_pool(name="psA", bufs=2, space="PSUM") as psA,
                tc.tile_pool(name="psB", bufs=4, space="PSUM") as psB,
            ):
                for e in range(E):
                    hT = htp.tile([128, FC, T], BF16, tag="hT")   # 64 KB/part
                    # stage A: hT[f, t] = gelu(W1[e].T-chunks @ xT + b1)
                    for fc in range(FC):
                        w1_t = w1p.tile([128, HC, 128], BF16, tag="w1")
                        nc.sync.dma_start(out=w1_t, in_=w1_d[e, fc])
                        pa = psA.tile([128, T], F32, tag="pa")
                        for hc in range(HC):
                            for nwin in range(2):
                                nc.tensor.matmul(
                                    pa[:, nwin * 512 : (nwin + 1) * 512],
                                    lhsT=w1_t[:, hc, :],
                                    rhs=xbf[:, hc, nwin * 512 : (nwin + 1) * 512],
                                    start=(hc == 0),
                                    stop=(hc == HC - 1),
                                )
                        nc.scalar.activation(
                            out=hT[:, fc, :], in_=pa, func=AF.Gelu,
                            bias=b1_sb[:, e, fc : fc + 1], scale=1.0,
                        )
                    # stage B: out_acc[:, c, :] += w[:, c, e] * (hT.T @ W2[e])
                    for tg in range(2):
                        for hh in range(2):
                            pbs = [
                                psB.tile([128, 512], F32, tag="pb") for _ in range(4)
                            ]
                            for fc in range(FC):
                                w2_t = w2p.tile([128, 512], BF16, tag="w2")
                                nc.sync.dma_start(
                                    out=w2_t,
                                    in_=w2_d[e, fc, :, hh * 512 : (hh + 1) * 512],
                                )
                                for ci in range(4):
                                    c = tg * 4 + ci
                                    nc.tensor.matmul(
                                        pbs[ci],
                                        lhsT=hT[:, fc, c * 128 : (c + 1) * 128],
                                        rhs=w2_t,
                                        start=(fc == 0),
                                        stop=(fc == FC - 1),
                                    )
                            for ci in range(4):
                                c = tg * 4 + ci
                                sl = slice(hh * 512, (hh + 1) * 512)
                                nc.vector.scalar_tensor_tensor(
                                    out=out_acc[:, c, sl],
                                    in0=pbs[ci],
                                    scalar=wsb[:, c, e : e + 1],
                                    in1=out_acc[:, c, sl],
                                    op0=ALU.mult,
                                    op1=ALU.add,
                                )

            nc.sync.dma_start(
                out=out_d.rearrange("(c p) h -> p c h", p=128), in_=out_acc
            )

    _split_multi_waits(nc)
    return nc


_NC_CACHE = None


def _get_nc():
    global _NC_CACHE
    if _NC_CACHE is None:
        _NC_CACHE = _build_nc()
    return _NC_CACHE


def _prep_inputs(x, Wg, bg, W1, b1, W2, b2):
    """Host-side sharding + layout prep. Returns per-core input maps."""
    xf = np.ascontiguousarray(x, dtype=np.float32).reshape(N, H)
    wg_r = np.ascontiguousarray(Wg, dtype=np.float32).reshape(HC, 128, E)
    bg_r = np.ascontiguousarray(bg, dtype=np.float32)
    # W1 [E, H, F] -> [E, FC, 128p(h), HC, 128(f)]
    w1_r = np.ascontiguousarray(
        np.asarray(W1, dtype=np.float32)
        .reshape(E, HC, 128, FC, 128)
        .transpose(0, 3, 2, 1, 4)
    ).astype(ml_dtypes.bfloat16)
    b1_r = np.ascontiguousarray(b1, dtype=np.float32).reshape(E, FC, 128)
    # W2 [E, F, H] -> [E, FC, 128(f), H]  (pure reshape)
    w2_r = np.ascontiguousarray(W2, dtype=np.float32).reshape(E, FC, 128, H).astype(
        ml_dtypes.bfloat16
    )
    b2_r = np.ascontiguousarray(b2, dtype=np.float32)

    in_maps = []
    for c in range(NCORES):
        xs = xf[c * T : (c + 1) * T]                       # [T, H]
        xt = np.ascontiguousarray(xs.T).reshape(HC, 128, T)
        in_maps.append(
            {
                "xt_bf": xt.astype(ml_dtypes.bfloat16),
                "xt_f32": xt,
                "wg": wg_r,
                "bg": bg_r,
                "w1": w1_r,
                "b1": b1_r,
                "w2": w2_r,
                "b2": b2_r,
            }
        )
    return in_maps


def kernel(x, Wg, bg, W1, b1, W2, b2):
    nc = _get_nc()
    in_maps = _prep_inputs(x, Wg, bg, W1, b1, W2, b2)
    res = run_bass_kernel_spmd(nc, in_maps, list(range(NCORES)), trace=False)
    out = np.concatenate([res.results[c]["out"] for c in range(NCORES)], axis=0)
    return out.reshape(B, S, H)


# revision 4
# speedup vs baseline: 1.4555x; 1.4555x over previous
"""MoE layer (dense all-expert compute + top-2 routing) on 8 TRN2 NeuronCores.

Sharding: token-parallel. x has N = 4*2048 = 8192 tokens; each core gets 1024
tokens and computes all 6 experts for them (the reference computes every
expert densely for every token, so expert-parallel would save nothing and
leave cores idle with E=6 on 8 cores). No collectives needed: each core
writes its own token-slice of the output.

Per-core device program (T=1024 tokens, H=1024, F=4096, E=6):
  gate  = x @ Wg + bg              (fp32 PE matmuls -> exact top-2 selection)
  w[t,e] = top-2 softmax weights scattered to expert slots (DVE mask ops)
  out   = sum_e w[:,e] * (gelu(x @ W1[e] + b1[e]) @ W2[e]) + w @ b2
Big matmuls run in bf16 (full PE rate, fp32 PSUM accumulation); the first
matmul produces hT in [F-partition, token] layout so the second needs no
transpose.
"""

import numpy as np
import ml_dtypes

import concourse.bass as bass
import concourse.mybir as mybir
from concourse.tile import TileContext
from concourse.bass_utils import run_bass_kernel_spmd
from concourse.masks import make_identity

F32 = mybir.dt.float32
BF16 = mybir.dt.bfloat16
AF = mybir.ActivationFunctionType
ALU = mybir.AluOpType

NCORES = 8
B, S, H, F, E = 4, 2048, 1024, 4096, 6
N = B * S                    # 8192 tokens
T = N // NCORES              # 1024 tokens per core
HC = H // 128                # 8 h-chunks
FC = F // 128                # 32 f-chunks
TC = T // 128                # 8 token tiles per core


def _split_multi_waits(nc, max_waits=1):
    """The walrus build in this env accepts only one sync-wait per
    instruction; hoist extra waits onto same-engine NOPs inserted before."""
    for f in nc.m.functions:
        for bb in f.blocks:
            new = []
            dirty = False
            for inst in bb.instructions:
                si = inst.sync_info
                waits = list(si.on_wait) if si else []
                if len(waits) > max_waits:
                    dirty = True
                    for j, w in enumerate(waits[max_waits:]):
                        nop = mybir.InstNoOp(
                            name=f"{inst.name}-wsplit{j}", ins=[], outs=[]
                        )
                        nop.engine = inst.engine
                        nop.sync_info = mybir.SyncInfo(on_wait=[w], on_update=[])
                        new.append(nop)
                    inst.sync_info = mybir.SyncInfo(
                        on_wait=waits[:max_waits], on_update=list(si.on_update)
                    )
                new.append(inst)
            if dirty:
                bb.instructions = new


def _build_nc():
    nc = bass.Bass("TRN2", target_bir_lowering=False, debug=False)

    xt_bf = nc.dram_tensor("xt_bf", [HC, 128, T], BF16, kind="ExternalInput")
    xt_f32 = nc.dram_tensor("xt_f32", [HC, 128, T], F32, kind="ExternalInput")
    wg_d = nc.dram_tensor("wg", [HC, 128, E], F32, kind="ExternalInput")
    bg_d = nc.dram_tensor("bg", [E], F32, kind="ExternalInput")
    w1_d = nc.dram_tensor("w1", [E, FC, 128, HC, 128], BF16, kind="ExternalInput")
    b1_d = nc.dram_tensor("b1", [128, E, FC], F32, kind="ExternalInput")
    w2_d = nc.dram_tensor("w2", [E, FC, 128, H], BF16, kind="ExternalInput")
    b2_d = nc.dram_tensor("b2", [E, H], F32, kind="ExternalInput")
    out_d = nc.dram_tensor("out", [T, H], F32, kind="ExternalOutput")

    with TileContext(nc) as tc:
        with (
            tc.tile_pool(name="const", bufs=1) as const,
            tc.tile_pool(name="acc", bufs=1) as accp,
        ):
            # resident tiles
            xbf = const.tile([128, HC, T], BF16)           # 16 KB/part
            nc.sync.dma_start(out=xbf, in_=xt_bf.rearrange("c p t -> p c t"))
            b1_sb = const.tile([128, E, FC], F32)          # 768 B/part
            nc.gpsimd.dma_start(out=b1_sb, in_=b1_d[:, :, :])
            bg_sb = const.tile([128, E], F32)
            nc.gpsimd.dma_start(
                out=bg_sb,
                in_=bass.AP(tensor=bg_d, offset=0, ap=[[0, 128], [1, E]]),
            )
            b2_sb = const.tile([E, H], F32)
            nc.gpsimd.dma_start(out=b2_sb, in_=b2_d[:, :])
            ident = const.tile([128, 128], F32)
            make_identity(nc, ident)
            wsb = const.tile([128, TC, E], F32)            # routing weights
            wt_sb = const.tile([E, T], F32)                # transposed routing weights
            out_acc = accp.tile([128, TC, H], F32)         # 32 KB/part

            # ---------------- gate + routing ----------------
            with (
                tc.tile_pool(name="gx", bufs=1) as gx,
                tc.tile_pool(name="gps", bufs=2, space="PSUM") as gps,
                tc.tile_pool(name="grt", bufs=4) as grt,
                tc.tile_pool(name="gwt", bufs=2, space="PSUM") as gwt,
            ):
                xf32 = gx.tile([128, HC, T], F32)          # 32 KB/part, freed after
                nc.sync.dma_start(out=xf32, in_=xt_f32.rearrange("c p t -> p c t"))
                wg_sb = grt.tile([128, HC, E], F32)
                nc.gpsimd.dma_start(out=wg_sb, in_=wg_d.rearrange("c p e -> p c e"))

                for c in range(TC):
                    gp = gps.tile([128, E], F32)
                    for hc in range(HC):
                        nc.tensor.matmul(
                            gp,
                            lhsT=xf32[:, hc, c * 128 : (c + 1) * 128],
                            rhs=wg_sb[:, hc, :],
                            start=(hc == 0),
                            stop=(hc == HC - 1),
                        )
                    g_t = grt.tile([128, E], F32)
                    nc.vector.tensor_add(g_t, gp, bg_sb)
                    m1 = grt.tile([128, 1], F32)
                    nc.vector.tensor_reduce(
                        m1, g_t, axis=mybir.AxisListType.X, op=ALU.max
                    )
                    is1 = grt.tile([128, E], F32)
                    nc.vector.tensor_scalar(is1, g_t, m1[:, :], None, op0=ALU.is_ge)
                    g2 = grt.tile([128, E], F32)
                    nc.vector.scalar_tensor_tensor(
                        out=g2, in0=is1, scalar=-1e30, in1=g_t,
                        op0=ALU.mult, op1=ALU.add,
                    )
                    m2 = grt.tile([128, 1], F32)
                    nc.vector.tensor_reduce(
                        m2, g2, axis=mybir.AxisListType.X, op=ALU.max
                    )
                    is2 = grt.tile([128, E], F32)
                    nc.vector.tensor_scalar(is2, g2, m2[:, :], None, op0=ALU.is_ge)
                    negm1 = grt.tile([128, 1], F32)
                    nc.vector.tensor_scalar_mul(negm1, m1, -1.0)
                    e2 = grt.tile([128, 1], F32)
                    nc.scalar.activation(
                        out=e2, in_=m2, func=AF.Exp, bias=negm1[:, :], scale=1.0
                    )
                    denom = grt.tile([128, 1], F32)
                    nc.vector.tensor_scalar_add(denom, e2, 1.0)
                    winv = grt.tile([128, 1], F32)
                    nc.vector.reciprocal(winv, denom)
                    w2nd = grt.tile([128, 1], F32)
                    nc.vector.tensor_mul(w2nd, e2, winv)
                    nc.vector.tensor_scalar_mul(wsb[:, c, :], is1, winv[:, :])
                    nc.vector.scalar_tensor_tensor(
                        out=wsb[:, c, :], in0=is2, scalar=w2nd[:, :],
                        in1=wsb[:, c, :], op0=ALU.mult, op1=ALU.add,
                    )
                    # transpose w tile -> wT [E, 128]
                    wt_ps = gwt.tile([E, 128], F32)
                    nc.tensor.transpose(wt_ps, wsb[:, c, :], ident)
                    nc.vector.tensor_copy(wt_sb[:, c * 128 : (c + 1) * 128], wt_ps)

                # out_acc init = w @ b2  (handles arbitrary b2; zeros in practice)
                for c in range(TC):
                    for half in range(2):
                        po = gps.tile([128, 512], F32)
                        nc.tensor.matmul(
                            po,
                            lhsT=wt_sb[:, c * 128 : (c + 1) * 128],
                            rhs=b2_sb[:, half * 512 : (half + 1) * 512],
                            start=True,
                            stop=True,
                        )
                        nc.scalar.activation(
                            out=out_acc[:, c, half * 512 : (half + 1) * 512],
                            in_=po, func=AF.Copy,
                        )

            # ---------------- main expert loop ----------------
            with (
                tc.tile_pool(name="w1p", bufs=3) as w1p,
                tc.tile_pool(name="w2p", bufs=4) as w2p,
                tc.tile_pool(name="ht", bufs=1) as htp,
                tc.tile_pool(name="psA", bufs=2, space="PSUM") as psA,
                tc.tile_pool(name="psB", bufs=4, space="PSUM") as psB,
            ):
                for e in range(E):
                    hT = htp.tile([128, FC, T], BF16, tag="hT")   # 64 KB/part
                    # stage A: hT[f, t] = gelu(W1[e].T-chunks @ xT + b1)
                    for fc in range(FC):
                        w1_t = w1p.tile([128, HC, 128], BF16, tag="w1")
                        nc.sync.dma_start(out=w1_t, in_=w1_d[e, fc])
                        pa = psA.tile([128, T], F32, tag="pa")
                        for hc in range(HC):
                            for nwin in range(2):
                                nc.tensor.matmul(
                                    pa[:, nwin * 512 : (nwin + 1) * 512],
                                    lhsT=w1_t[:, hc, :],
                                    rhs=xbf[:, hc, nwin * 512 : (nwin + 1) * 512],
                                    start=(hc == 0),
                                    stop=(hc == HC - 1),
                                )
                        nc.scalar.activation(
                            out=hT[:, fc, :], in_=pa, func=AF.Gelu,
                            bias=b1_sb[:, e, fc : fc + 1], scale=1.0,
                        )
                    # stage B: out_acc[:, c, :] += w[:, c, e] * (hT.T @ W2[e])
                    for tg in range(2):
                        for hh in range(2):
                            pbs = [
                                psB.tile([128, 512], F32, tag="pb", name=f"pb_{e}_{tg}_{hh}_{i}")
                                for i in range(4)
                            ]
                            for fc in range(FC):
                                w2_t = w2p.tile([128, 512], BF16, tag="w2")
                                nc.sync.dma_start(
                                    out=w2_t,
                                    in_=w2_d[e, fc, :, hh * 512 : (hh + 1) * 512],
                                )
                                for ci in range(4):
                                    c = tg * 4 + ci
                                    nc.tensor.matmul(
                                        pbs[ci],
                                        lhsT=hT[:, fc, c * 128 : (c + 1) * 128],
                                        rhs=w2_t,
                                        start=(fc == 0),
                                        stop=(fc == FC - 1),
                                    )
                            for ci in range(4):
                                c = tg * 4 + ci
                                sl = slice(hh * 512, (hh + 1) * 512)
                                nc.vector.scalar_tensor_tensor(
                                    out=out_acc[:, c, sl],
                                    in0=pbs[ci],
                                    scalar=wsb[:, c, e : e + 1],
                                    in1=out_acc[:, c, sl],
                                    op0=ALU.mult,
                                    op1=ALU.add,
                                )

            nc.sync.dma_start(
                out=out_d.rearrange("(c p) h -> p c h", p=128), in_=out_acc
            )

    _split_multi_waits(nc)
    return nc


_NC_CACHE = None


def _get_nc():
    global _NC_CACHE
    if _NC_CACHE is None:
        _NC_CACHE = _build_nc()
    return _NC_CACHE


def _prep_inputs(x, Wg, bg, W1, b1, W2, b2):
    """Host-side sharding + layout prep. Returns per-core input maps."""
    xf = np.ascontiguousarray(x, dtype=np.float32).reshape(N, H)
    wg_r = np.ascontiguousarray(Wg, dtype=np.float32).reshape(HC, 128, E)
    bg_r = np.ascontiguousarray(bg, dtype=np.float32)
    # W1 [E, H, F] -> [E, FC, 128p(h), HC, 128(f)]
    w1_r = np.ascontiguousarray(
        np.asarray(W1, dtype=np.float32)
        .reshape(E, HC, 128, FC, 128)
        .transpose(0, 3, 2, 1, 4)
    ).astype(ml_dtypes.bfloat16)
    b1_r = np.ascontiguousarray(
        np.asarray(b1, dtype=np.float32).reshape(E, FC, 128).transpose(2, 0, 1)
    )
    # W2 [E, F, H] -> [E, FC, 128(f), H]  (pure reshape)
    w2_r = np.ascontiguousarray(W2, dtype=np.float32).reshape(E, FC, 128, H).astype(
        ml_dtypes.bfloat16
    )
    b2_r = np.ascontiguousarray(b2, dtype=np.float32)

    in_maps = []
    for c in range(NCORES):
        xs = xf[c * T : (c + 1) * T]                       # [T, H]
        xt = np.ascontiguousarray(xs.T).reshape(HC, 128, T)
        in_maps.append(
            {
                "xt_bf": xt.astype(ml_dtypes.bfloat16),
                "xt_f32": xt,
                "wg": wg_r,
                "bg": bg_r,
                "w1": w1_r,
                "b1": b1_r,
                "w2": w2_r,
                "b2": b2_r,
            }
        )
    return in_maps


def kernel(x, Wg, bg, W1, b1, W2, b2):
    nc = _get_nc()
    in_maps = _prep_inputs(x, Wg, bg, W1, b1, W2, b2)
    res = run_bass_kernel_spmd(nc, in_maps, list(range(NCORES)), trace=False)
    out = np.concatenate([res.results[c]["out"] for c in range(NCORES)], axis=0)
    return out.reshape(B, S, H)
